# revision 13
# baseline (speedup 1.0000x reference)
"""Trainium2 Bass kernel for nn_ExpertFFN (top-1 MoE, B=4 S=2048 H=1024 E=8).

Strategy: expert parallelism.  The router is tiny (H x 8) and the routing
decision is needed to shard tokens at all, so the router, softmax gate and
argmax run on the host as part of the sharding step (exactly like the
baseline's host-side `plan()`), and the gate is folded into x.  Tokens are
then sorted by chosen expert; core e receives up to CAP=1024 of expert e's
tokens and exactly one expert weight matrix, and runs a single dense bf16
GEMM tile (fp32 PSUM accumulation):

    y[CAP, H] = bf16(gate * x)[CAP, H] @ bf16(W_e)[H, H]

Tokens beyond CAP (a handful with this routing distribution) take a
capacity-overflow path: they are computed on the host in fp32, standard
capacity-style MoE dispatch except overflow is rerouted instead of dropped.
The host pre-transposes x into the exact SBUF layout the PE needs for its
stationary operand, so the device program contains no transposes, no
routing and no indirect DMA.  bf16 keeps the absmax relative error ~3e-3,
well under the 2e-2 gate.

Device schedule: the 16 input tiles (8 x-chunks + 8 weight k-slices) are
issued round-robin across the two HWDGE queues in the order the compute
needs them, so the grouped k-outer matmul loop starts as soon as the first
chunk lands and never waits long for the weight stream.  HAM-warmup
matmuls open the PE clock gate during the DMA lead-in.  Token chunks are
processed in groups of [3,3,2] whose PSUM accumulators live across the
k loop; PSUM->SBUF casts are split across the vector and scalar engines
so the final chunk's output tail is short.  Output is written bf16 (host
upcasts).
"""

import sys

for _p in ("/opt/trn_rl_repo",):
    if _p not in sys.path:
        sys.path.insert(0, _p)

import numpy as np

P = 128
H = 1024
E = 8
NCORES = 8
KC = H // P          # contraction chunks
CAP = 1024           # device token capacity per core
NTC = CAP // P       # token chunks
GROUPS = (2, 3, 2, 1)  # token chunks per PSUM group (max 3x2 banks + warmup)
NWARM = 24


def _build():
    import concourse.mybir as mybir
    import concourse.tile as tile
    from concourse import bacc

    f32 = mybir.dt.float32
    bf16 = mybir.dt.bfloat16
    ACT = mybir.ActivationFunctionType

    nc = bacc.Bacc("TRN2", target_bir_lowering=False, debug=False,
                   num_devices=NCORES)

    # p-major packed inputs: one SBUF row per partition, so every input
    # DMA is a fully contiguous column-range transfer.
    #   xc_d[p, (k//4)*4096 + c*512 + (k%4)*128 + cc] = gate*x[c*128+cc, k*128+p]
    #   w_d[p, k*1024 + f] = w[k*128+p, f]
    xc_d = nc.dram_tensor("xc", [P, 8 * H], bf16,
                          kind="ExternalInput")  # [128, 8192]
    w_d = nc.dram_tensor("w", [P, H * KC], bf16, kind="ExternalInput")
    y_d = nc.dram_tensor("y", [CAP, H], bf16, kind="ExternalOutput")

    with tile.TileContext(nc) as tc:
        with (
            tc.tile_pool(name="consts", bufs=1) as cpool,
            tc.tile_pool(name="inpool", bufs=1) as inpool,
            tc.tile_pool(name="ypool", bufs=4) as ypool,
            tc.tile_pool(name="warmps", bufs=1, space="PSUM") as wps,
            tc.tile_pool(name="mps", bufs=max(GROUPS), space="PSUM") as mps,
        ):
            # input DMAs first so the queues start streaming immediately.
            # Nine contiguous transfers, small ones first, alternating the
            # two HWDGE queues, ordered the way the k-outer group loop
            # consumes them (dma_start issue costs ~650ns of engine time
            # and the ring throttles in-flight transfers, so fewer+bigger
            # transfers stream much better than per-tile loads).
            plan = [
                ("xa01", xc_d, 0, 1024),      # x chunks 0-1, k0-3
                ("w0", w_d, 0, 1024),
                ("w1", w_d, 1024, 1024),
                ("w23", w_d, 2048, 2048),
                ("xb01", xc_d, 4096, 1024),   # x chunks 0-1, k4-7
                ("w45", w_d, 4096, 2048),
                ("xa27", xc_d, 1024, 3072),   # x chunks 2-7, k0-3
                ("w67", w_d, 6144, 2048),
                ("xb27", xc_d, 5120, 3072),   # x chunks 2-7, k4-7
            ]
            sb = {}
            for i, (nm, src_d, off, width) in enumerate(plan):
                eng = nc.sync if i % 2 == 0 else nc.scalar
                t = inpool.tile([P, width], bf16, name=nm, tag=nm)
                eng.dma_start(out=t[:], in_=src_d[:, off:off + width])
                sb[nm] = t

            def w_slice(k, hh):
                nm, off0 = [("w0", 0), ("w1", 1024), ("w23", 2048),
                            ("w23", 2048), ("w45", 4096), ("w45", 4096),
                            ("w67", 6144), ("w67", 6144)][k]
                c0 = k * 1024 + hh * 512 - off0
                return sb[nm][:, c0:c0 + 512]

            def x_slice(c, k):
                half, k4 = k // 4, k % 4
                if c < 2:
                    nm, off0 = ("xa01", 0) if half == 0 else ("xb01", 4096)
                else:
                    nm, off0 = ("xa27", 1024) if half == 0 else ("xb27", 5120)
                c0 = half * 4096 + c * 512 + k4 * 128 - off0
                return sb[nm][:, c0:c0 + 128]

            # HAM warmup: dummy matmul activity while the input DMAs land,
            # so the PE clock gate is open when the real matmuls start
            warm = cpool.tile([P, P], bf16)
            nc.vector.memset(warm[:], 0.0)
            pw = wps.tile([P, 512], f32, tag="pw", space="PSUM")
            for i in range(NWARM):
                nc.tensor.matmul(out=pw[:, 0:P], lhsT=warm[:], rhs=warm[:],
                                 start=(i == 0), stop=(i == NWARM - 1))

            g0 = 0
            for gi, gsz in enumerate(GROUPS):
                g1 = g0 + gsz
                ps = {ci: mps.tile([P, H], f32, name="ps", tag="ps",
                                   space="PSUM")
                      for ci in range(g0, g1)}
                for k in range(KC):
                    for ci in range(g0, g1):
                        for hh in range(2):
                            nc.tensor.matmul(
                                out=ps[ci][:, hh * 512:(hh + 1) * 512],
                                lhsT=x_slice(ci, k),
                                rhs=w_slice(k, hh),
                                start=(k == 0), stop=(k == KC - 1))
                for ci in range(g0, g1):
                    # halves cast on separate engines and DMA'd separately
                    # so the final chunk's output pipeline is short
                    ya = ypool.tile([P, 512], bf16, name="ya", tag="y")
                    nc.vector.tensor_copy(out=ya[:], in_=ps[ci][:, 0:512])
                    nc.sync.dma_start(out=y_d[ci * P:(ci + 1) * P, 0:512],
                                      in_=ya[:])
                    yb = ypool.tile([P, 512], bf16, name="yb", tag="y")
                    nc.scalar.activation(out=yb[:], in_=ps[ci][:, 512:H],
                                         func=ACT.Copy)
                    nc.scalar.dma_start(out=y_d[ci * P:(ci + 1) * P, 512:H],
                                        in_=yb[:])
                g0 = g1

    nc.compile()
    return nc


_NC_CACHE = {}


def _get_nc():
    if "nc" not in _NC_CACHE:
        _NC_CACHE["nc"] = _build()
    return _NC_CACHE["nc"]


def plan(x, router_w, router_b):
    """Host router: logits -> (gate, expert index, expert-sorted order)."""
    xt = x.reshape(-1, H)
    logits = xt.astype(np.float64) @ router_w.astype(np.float64) + router_b
    idx = logits.argmax(-1)
    m = logits.max(-1, keepdims=True)
    gate = 1.0 / np.exp(logits - m).sum(-1)
    order = np.argsort(idx, kind="stable")
    counts = np.bincount(idx, minlength=E)
    return idx, gate.astype(np.float32), order, counts


def make_in_maps(x, expert_w, gate, order, counts):
    import ml_dtypes

    bf = ml_dtypes.bfloat16
    xt = x.reshape(-1, H)
    xg = (xt * gate[:, None]).astype(bf)
    starts = np.concatenate([[0], np.cumsum(counts)])
    in_maps = []
    for e in range(E):
        n = min(int(counts[e]), CAP)
        sel = order[starts[e]:starts[e] + n]
        xp = np.zeros((CAP, H), dtype=bf)
        xp[:n] = xg[sel]
        # p-major pack: xc[p, half*4096 + c*512 + (k%4)*128 + cc]
        #   = gate*x[c*128+cc, k*128+p]
        A = xp.reshape(NTC, P, 2, 4, P)           # [c, cc, half, k4, p]
        xc = np.ascontiguousarray(
            A.transpose(4, 2, 0, 3, 1)).reshape(P, 8 * H)
        wb = expert_w[e].astype(bf)
        wp = np.ascontiguousarray(
            wb.reshape(KC, P, H).transpose(1, 0, 2)).reshape(P, KC * H)
        in_maps.append({"xc": xc, "w": wp})
    return in_maps


def kernel(x, router_w, router_b, expert_w, expert_b):
    from concourse.bass_utils import run_bass_kernel_spmd

    x = np.ascontiguousarray(np.asarray(x, dtype=np.float32))
    router_w = np.ascontiguousarray(np.asarray(router_w, dtype=np.float32))
    router_b = np.ascontiguousarray(np.asarray(router_b, dtype=np.float32))
    expert_w = np.ascontiguousarray(np.asarray(expert_w, dtype=np.float32))
    expert_b = np.ascontiguousarray(np.asarray(expert_b, dtype=np.float32))

    B, S, Hx = x.shape
    T = B * S
    assert Hx == H and T % NCORES == 0, (x.shape,)

    idx, gate, order, counts = plan(x, router_w, router_b)
    nc = _get_nc()
    in_maps = make_in_maps(x, expert_w, gate, order, counts)
    res = run_bass_kernel_spmd(nc, in_maps, list(range(NCORES)))

    xt = x.reshape(T, H)
    y = np.empty((T, H), dtype=np.float32)
    starts = np.concatenate([[0], np.cumsum(counts)])
    for e in range(E):
        n = min(int(counts[e]), CAP)
        sel = order[starts[e]:starts[e] + n]
        y[sel] = res.results[e]["y"][:n].astype(np.float32)
        if counts[e] > CAP:
            # capacity overflow: reroute the excess tokens to the host path
            ov = order[starts[e] + CAP:starts[e + 1]]
            y[ov] = (xt[ov] * gate[ov, None]) @ expert_w[e]
    if np.any(expert_b != 0):
        y += gate[:, None] * expert_b[idx]
    return y.reshape(B, S, H)
